# revision 1
# baseline (speedup 1.0000x reference)
"""Bass/Trainium2 kernel for a 2-layer bidirectional LSTM (CustomBiLSTM).

Strategy: data-parallel over batch across 8 NeuronCores (B=64 -> 8 per core).
Per core, each layer runs its forward and backward recurrent chains
concurrently (independent), staggered to hide the per-step serial
dependency chain (matmul -> sigmoid -> cell update -> tanh -> h -> matmul).
The wall time is bound by that latency chain, not engine throughput.

Layout is fully transposed: features on SBUF partitions, batch on the free
dim.  Gate pre-activations for a window of 16 timesteps live in one PSUM
bank as [128 gate-features, 4 gates x 16 steps x 8 batch]; the input
projection (Wih @ x) plus the bias (as a K=1 rank-1 matmul against a ones
row) are precomputed into the bank off the critical path at reduced
scheduler priority, and the tiny recurrent matmuls (Whh_g @ h, one per
gate) accumulate into it each step.

The g-gate weights are pre-scaled by 2 on the host so a single Sigmoid
activation covers all 4 gates (tanh(z) = 2*sigmoid(2z) - 1); the affine
fix-up is fused into the DVE cell-state update via scalar_tensor_tensor.
"""

import numpy as np
import ml_dtypes

try:
    import concourse.bass as bass
except ImportError:
    import sys
    sys.path.insert(0, "/opt/trn_rl_repo")
    import concourse.bass as bass

import concourse.bacc as bacc
import concourse.tile as tile
from concourse import mybir
from concourse.bass_utils import run_bass_kernel_spmd

F32 = mybir.dt.float32
BF16 = mybir.dt.bfloat16
AF = mybir.ActivationFunctionType
ALU = mybir.AluOpType
BF16_NP = ml_dtypes.bfloat16

H = 128          # hidden dim
D = 128          # input dim
B = 64           # global batch
T = 1024         # sequence length
NCORES = 8
BL = B // NCORES  # per-core batch = 8
G = 4            # gates (i, f, g, o)
SH = 1           # batch sub-shards per direction (chains per dir)
BS = BL // SH    # batch per chain
CH = 512 // (G * BS)  # window steps (G*CH*BS = 512 = one bank)

DIRS = ("a", "b")          # a = forward, b = backward
CHAINS = [(dn, s) for dn in DIRS for s in range(SH)]


def build_program(t_len=T, debug_taps=False, repeat=None):
    nw = t_len // CH
    nc = bacc.Bacc("TRN2", target_bir_lowering=False, debug=False)

    # ---- DRAM I/O ----
    xT_d = nc.dram_tensor("xT", [D, SH * t_len * BS], BF16, kind="ExternalInput")
    whh_d, wih_d, bias_d = {}, {}, {}
    for lay in (1, 2):
        for dirn in DIRS:
            cell = f"{dirn}{lay}"
            whh_d[cell] = nc.dram_tensor(f"whhT_{cell}", [H, G * H], BF16,
                                         kind="ExternalInput")
            bias_d[cell] = nc.dram_tensor(f"bias_{cell}", [1, G * H], BF16,
                                          kind="ExternalInput")
            nchunk = 1 if lay == 1 else 2
            wih_d[cell] = [
                nc.dram_tensor(f"wihT_{cell}_{q}", [H, G * H], BF16,
                               kind="ExternalInput")
                for q in range(nchunk)
            ]
    o2_d = {dirn: nc.dram_tensor(f"o2{dirn}", [H, SH * t_len * BS], BF16,
                                 kind="ExternalOutput")
            for dirn in DIRS}
    o1_d = None
    if debug_taps:
        o1_d = {dirn: nc.dram_tensor(f"o1{dirn}", [H, SH * t_len * BS], BF16,
                                     kind="ExternalOutput")
                for dirn in DIRS}

    with tile.TileContext(nc) as tc:
        with tc.tile_pool(name="const", bufs=1) as const, \
             tc.tile_pool(name="ps", bufs=1, space="PSUM") as psp, \
             tc.tile_pool(name="work", bufs=4) as work:

            # ---- persistent SBUF ----
            xT = const.tile([D, SH * t_len * BS], BF16, tag="xT")
            ndma = 8
            chunk = (SH * t_len * BS) // ndma
            for i in range(ndma):
                nc.sync.dma_start(out=xT[:, i * chunk:(i + 1) * chunk],
                                  in_=xT_d.ap()[:, i * chunk:(i + 1) * chunk])

            whh_s, wih_s, bias_s = {}, {}, {}
            for cell in whh_d:
                whh_s[cell] = const.tile([H, G * H], BF16, name=f"whh_{cell}")
                nc.sync.dma_start(out=whh_s[cell][:, :], in_=whh_d[cell].ap()[:, :])
                bias_s[cell] = const.tile([1, G * H], BF16, name=f"bias_{cell}")
                nc.sync.dma_start(out=bias_s[cell][:, :], in_=bias_d[cell].ap()[:, :])
                wih_s[cell] = []
                for q, dd in enumerate(wih_d[cell]):
                    wt = const.tile([H, G * H], BF16, name=f"wih_{cell}_{q}")
                    nc.sync.dma_start(out=wt[:, :], in_=dd.ap()[:, :])
                    wih_s[cell].append(wt)

            ones_row = const.tile([1, CH * BS], BF16, tag="ones_row")
            nc.vector.memset(ones_row[:, :], 1.0)

            # h buffers per chain (bf16): layer1 feeds layer2; layer2 is output
            h1 = {ck: const.tile([H, t_len * BS], BF16, name=f"h1{ck[0]}{ck[1]}")
                  for ck in CHAINS}
            h2 = {ck: const.tile([H, t_len * BS], BF16, name=f"h2{ck[0]}{ck[1]}")
                  for ck in CHAINS}

            # 8 psum window banks: (chain, parity)
            psb = {(ck, p): psp.tile([H, G * CH * BS], F32,
                                     name=f"ps_{ck[0]}{ck[1]}{p}")
                   for ck in CHAINS for p in (0, 1)}

            class Chain:
                def __init__(self, lay, ck):
                    self.ck = ck
                    dirn, s = ck
                    self.key = f"{ck[0]}{ck[1]}"
                    cell = f"{dirn}{lay}"
                    self.whh = whh_s[cell]
                    self.wih = wih_s[cell]
                    self.bias = bias_s[cell]
                    if lay == 1:
                        self.rhs_src = [(xT, s * t_len * BS)]
                    else:
                        self.rhs_src = [(h1[("a", s)], 0), (h1[("b", s)], 0)]
                    self.hout = h1[ck] if lay == 1 else h2[ck]
                    self.fwd = (dirn == "a")
                    self.c_prev = None

                def tau(self, k):
                    return k if self.fwd else t_len - 1 - k

                def precompute(self, j):
                    """Fill psum window for time-block j with Wih@x + bias."""
                    ps = psb[(self.ck, j % 2)]
                    nq = len(self.rhs_src)
                    w = CH * BS
                    with tc.high_priority(offset=-1_000_000):
                        for g in range(G):
                            for q, (src, base) in enumerate(self.rhs_src):
                                nc.tensor.matmul(
                                    ps[:, g * w:(g + 1) * w],
                                    self.wih[q][:, g * H:(g + 1) * H],
                                    src[:, base + j * w:base + (j + 1) * w],
                                    start=(g == 0 and q == 0), stop=False)
                        # bias add as K=1 rank-1 matmul: bias_row^T @ ones_row
                        for g in range(G):
                            nc.tensor.matmul(
                                ps[:, g * w:(g + 1) * w],
                                self.bias[:, g * H:(g + 1) * H],
                                ones_row[:, :],
                                start=False, stop=(g == G - 1))

                def step(self, k):
                    tau = self.tau(k)
                    j, slot = tau // CH, tau % CH
                    ps = psb[(self.ck, j % 2)]
                    if k > 0:
                        tprev = tau - 1 if self.fwd else tau + 1
                        hprev = self.hout[:, tprev * BS:(tprev + 1) * BS]
                        for g in range(G):
                            off = g * CH * BS + slot * BS
                            # accumulate onto the precomputed Wih@x+bias; the
                            # psum group was closed by precompute(), so skip
                            # the sim's group bookkeeping (per-byte
                            # pending-zero state drives accumulate-vs-write)
                            nc.tensor.matmul(
                                ps[:, off:off + BS],
                                self.whh[:, g * H:(g + 1) * H],
                                hprev, start=False, stop=False,
                                skip_group_check=True)
                    s = work.tile([H, G * BS], F32, name=f"s{self.key}")
                    ps_view = ps[:, :].rearrange("p (g t b) -> p g t b",
                                                 g=G, t=CH)[:, :, slot, :]
                    s_view = s[:, :].rearrange("p (g b) -> p g b", g=G)
                    nc.scalar.activation(s_view, ps_view, AF.Sigmoid)

                    si, sf = s[:, 0:BS], s[:, BS:2 * BS]
                    s2g, so = s[:, 2 * BS:3 * BS], s[:, 3 * BS:4 * BS]
                    m2 = work.tile([H, BS], F32, name=f"m2{self.key}")
                    # m2 = (sigma(2g)-0.5) * sigma(i) = 0.5 * i_gate * tanh(g)
                    nc.vector.scalar_tensor_tensor(m2, s2g, 0.5, si,
                                                   ALU.subtract, ALU.mult)
                    c = work.tile([H, BS], F32, name=f"c{self.key}")
                    if k > 0:
                        # m1 on GPSIMD in parallel with m2 on DVE: the c-op
                        # waits max(m1, m2) instead of their serial sum
                        m1 = work.tile([H, BS], F32, name=f"m1{self.key}")
                        nc.gpsimd.tensor_tensor(m1, sf, self.c_prev, ALU.mult)
                        nc.vector.scalar_tensor_tensor(c, m2, 2.0, m1,
                                                       ALU.mult, ALU.add)
                    else:
                        nc.vector.tensor_scalar_mul(c, m2, 2.0)
                    self.c_prev = c
                    th = work.tile([H, BS], F32, name=f"th{self.key}")
                    nc.scalar.activation(th, c, AF.Tanh)
                    nc.vector.tensor_tensor(
                        self.hout[:, tau * BS:(tau + 1) * BS], so, th, ALU.mult)

                def blocks(self):
                    return list(range(nw)) if self.fwd else \
                        list(range(nw - 1, -1, -1))

            import contextlib
            loop_cm = tc.For_i(0, repeat, 1) if repeat else contextlib.nullcontext()
            with loop_cm:
              for lay in (1, 2):
                  if lay == 2 and o1_d is not None:
                      for (dn, s), t_ in h1.items():
                          nc.sync.dma_start(
                              out=o1_d[dn].ap()[:, s * t_len * BS:
                                                (s + 1) * t_len * BS],
                              in_=t_[:, :])
                  chains = [Chain(lay, ck) for ck in CHAINS]
                  for ch_ in chains:
                      ch_.precompute(ch_.blocks()[0])
                  for k in range(t_len):
                      for ch_ in chains:
                          ch_.step(k)
                      if k % CH == CH // 2 - 1:
                          nb = k // CH + 1
                          if nb < nw:
                              for ch_ in chains:
                                  ch_.precompute(ch_.blocks()[nb])
                      if lay == 2:
                          # stream finished output chunks (1/8th each)
                          ock = t_len // 8
                          if (k + 1) % ock == 0:
                              ci = k // ock
                              for ch_ in chains:
                                  dn, s = ch_.ck
                                  tlo = (ci if ch_.fwd else 7 - ci) * ock
                                  lo = tlo * BS
                                  nc.sync.dma_start(
                                      out=o2_d[dn].ap()[
                                          :, s * t_len * BS + lo:
                                          s * t_len * BS + lo + ock * BS],
                                      in_=h2[ch_.ck][:, lo:lo + ock * BS])

    nc.compile()
    return nc


def _prep_weights(Wih, Whh, bih, bhh):
    """Host-side weight massaging: transpose, gate-scale (g-gate x2), bf16."""
    gscale = np.array([1.0, 1.0, 2.0, 1.0], np.float32)
    fourh, ind = Wih.shape
    wihT = np.ascontiguousarray(Wih.T).astype(np.float32)      # [in, 4H]
    whhT = np.ascontiguousarray(Whh.T).astype(np.float32)      # [H, 4H]
    bias = (bih + bhh).astype(np.float32)                      # [4H]
    for g in range(G):
        sl = slice(g * H, (g + 1) * H)
        wihT[:, sl] *= gscale[g]
        whhT[:, sl] *= gscale[g]
        bias[sl] *= gscale[g]
    nq = ind // H
    wih_chunks = [np.ascontiguousarray(wihT[q * H:(q + 1) * H]).astype(BF16_NP)
                  for q in range(nq)]
    bias_row = bias.reshape(1, G * H).astype(BF16_NP)           # [1, 4H]
    return wih_chunks, whhT.astype(BF16_NP), bias_row


def core_xT(xs, t_len):
    """Per-core input layout: xs [BL, T, D] -> [D, SH*T*BS] shard-major."""
    parts = []
    for s in range(SH):
        xb = xs[s * BS:(s + 1) * BS]                   # [BS, T, D]
        parts.append(xb.transpose(2, 1, 0).reshape(D, t_len * BS))
    return np.ascontiguousarray(np.concatenate(parts, axis=1)).astype(BF16_NP)


def core_gather(res_c, t_len):
    """Per-core output: {'o2a','o2b'} [H, SH*T*BS] -> [BL, T, 2H] fp32."""
    out = np.empty((BL, t_len, 2 * H), np.float32)
    for dn, off in (("a", 0), ("b", H)):
        o = np.asarray(res_c[f"o2{dn}"]).astype(np.float32)
        for s in range(SH):
            blk = o[:, s * t_len * BS:(s + 1) * t_len * BS]
            out[s * BS:(s + 1) * BS, :, off:off + H] = \
                blk.reshape(H, t_len, BS).transpose(2, 1, 0)
    return out


_PROG_CACHE = {}


def prepare_in_maps(x, kw):
    x = np.asarray(x, np.float32)
    t_len = x.shape[1]
    cells = {"a1": (kw["Wih_fw1"], kw["Whh_fw1"], kw["bih_fw1"], kw["bhh_fw1"]),
             "b1": (kw["Wih_bw1"], kw["Whh_bw1"], kw["bih_bw1"], kw["bhh_bw1"]),
             "a2": (kw["Wih_fw2"], kw["Whh_fw2"], kw["bih_fw2"], kw["bhh_fw2"]),
             "b2": (kw["Wih_bw2"], kw["Whh_bw2"], kw["bih_bw2"], kw["bhh_bw2"])}
    wmaps = {}
    for cell, (Wih, Whh, bih, bhh) in cells.items():
        wih_chunks, whhT, bias_row = _prep_weights(
            np.asarray(Wih, np.float32), np.asarray(Whh, np.float32),
            np.asarray(bih, np.float32), np.asarray(bhh, np.float32))
        wmaps[f"whhT_{cell}"] = whhT
        wmaps[f"bias_{cell}"] = bias_row
        for q, wc in enumerate(wih_chunks):
            wmaps[f"wihT_{cell}_{q}"] = wc

    core_ids = list(range(NCORES))
    in_maps = []
    for c in core_ids:
        m = {"xT": core_xT(x[c * BL:(c + 1) * BL], t_len)}
        m.update(wmaps)
        in_maps.append(m)
    return in_maps, core_ids


def kernel(x, lengths, **kw):
    x = np.asarray(x, np.float32)
    t_len = x.shape[1]
    in_maps, core_ids = prepare_in_maps(x, kw)
    if t_len not in _PROG_CACHE:
        _PROG_CACHE[t_len] = build_program(t_len)
    nc = _PROG_CACHE[t_len]
    return _execute(nc, in_maps, core_ids, t_len)[0]


def _execute(nc, in_maps, core_ids, t_len, **run_kwargs):
    r = run_bass_kernel_spmd(nc, in_maps, core_ids, **run_kwargs)
    out = np.empty((B, t_len, 2 * H), np.float32)
    for c in core_ids:
        out[c * BL:(c + 1) * BL] = core_gather(r.results[c], t_len)
    return out, r



# revision 15
# speedup vs baseline: 1.1340x; 1.1340x over previous
"""Bass/Trainium2 kernel for a 2-layer bidirectional LSTM (CustomBiLSTM).

Strategy: data-parallel over batch across 8 NeuronCores (B=64 -> 8 per core).
Per core, each layer runs its forward and backward chains MERGED: both
chains are step-indexed (chain b's step k processes original time T-1-k, with
all time reversal handled by negative-stride access patterns), so every
per-step instruction covers both chains at once:

  - 8 recurrent matmuls (2 chains x 4 gates) accumulate into one PSUM bank
    laid out [slot(8), chain(2), gate(4), batch(8)] = 512 f32 columns.
  - ONE Activation instruction applies exact sigmoid to the whole 64-column
    slot (g-gate weights pre-scaled by 2: sigma(2z) = (tanh(z)+1)/2).
  - A short chain of Vector-engine ops (same engine -> no semaphore hops)
    computes the cell update, including two custom DVE ops:
      CFMA:    c = clip(2*m2 + m1, -Bc, Bc)
      TANH_P7: tanh(c) as a degree-7 odd Horner polynomial (c pre-clamped)
    and writes h (bf16) into a step-indexed h buffer.

The critical cycle per step is matmul -> sigmoid -> 5 DVE ops -> matmul.
Input projections (Wih @ x + bias) are precomputed into PSUM windows of 8
steps, emitted in small pieces spread across the previous window's steps so
the in-order PE queue never stalls a recurrent matmul.
"""

import numpy as np
import ml_dtypes

try:
    import concourse.bass as bass
except ImportError:
    import sys
    sys.path.insert(0, "/opt/trn_rl_repo")
    import concourse.bass as bass

import concourse.bacc as bacc
import concourse.tile as tile
import concourse.ap as cap
from concourse import mybir
from concourse.bass_utils import run_bass_kernel_spmd

# ---------------------------------------------------------------------------
# Custom DVE op registration
# ---------------------------------------------------------------------------
from concourse.dve_ops import DveOp, OPS, CUSTOM_DVE_SPECS, _SUB_OPCODE_FOR_NAME
from concourse.dve_spec import (
    Spec, Src0, Src1, C0, C1, C2, Zero, One, maxx, minn, lower, _has_src1,
)
from concourse.dve_uop import DveOpSpec


def _register(name, spec):
    if name in _SUB_OPCODE_FOR_NAME:
        return next(o for o in OPS if o.name == name)
    row = max(_SUB_OPCODE_FOR_NAME.values()) + 1
    assert row < 0x20, "custom DVE opcode rows exhausted"
    _SUB_OPCODE_FOR_NAME[name] = row
    shas = {}
    for ver in ("v3", "v4"):
        try:
            tmp = DveOpSpec(name=name, opcode=row, uops=lower(spec, ver=ver),
                            rd1_en=_has_src1(spec))
            shas[ver] = tmp.sha(ver)
        except Exception:
            pass
    op = DveOp(name, spec, subdim=False, uops_sha=shas)
    OPS.append(op)
    CUSTOM_DVE_SPECS[name] = spec
    return op


# c = clip(in0*s0 + in1, -s1, s1)
CFMA = _register(
    "ANT_CFMA",
    Spec(body=maxx(minn(Src0 * C0 + Src1, C1), Zero - C1),
         reference=lambda in0, in1, s0, s1, imm2:
         np.clip(in0 * s0 + in1, -s1, s1)),
)

# tanh(x) ~= x * (1 + s1... deg-7 odd Horner: ((s0*u + s1)*u + imm2)*u + 1
_u = Src0 * Src0
TANH_P7 = _register(
    "ANT_TANH_P7",
    Spec(body=((((C0 * _u + C1) * _u + C2) * _u) + One) * Src0,
         reference=lambda in0, s0, s1, imm2:
         ((((s0 * (in0 * in0) + s1) * (in0 * in0) + imm2) * (in0 * in0))
          + 1.0) * in0),
)

# h = tanh_deg5(in0) * in1: out = (((s0*u + s1)*u + imm2) * in0) * in1
D1H = _register(
    "ANT_D1H",
    Spec(body=(((C0 * _u + C1) * _u + C2) * Src0) * Src1,
         reference=lambda in0, in1, s0, s1, imm2:
         (((s0 * (in0 * in0) + s1) * (in0 * in0) + imm2) * in0) * in1),
)

# h = tanh_deg7(c) * sigma_o with w := c*sigma_o passed as in1:
#   out = ((((s0*u + s1)*u + imm2)*u) + 1) * in1,  u = in0^2
# (equals x*(1 + q3 u + q5 u^2 + q7 u^3) * sigma_o since in1 = x*sigma_o)
TANHH = _register(
    "ANT_TANHH",
    Spec(body=((((C0 * _u + C1) * _u + C2) * _u) + One) * Src1,
         reference=lambda in0, in1, s0, s1, imm2:
         ((((s0 * (in0 * in0) + s1) * (in0 * in0) + imm2) * (in0 * in0))
          + 1.0) * in1),
)

F32 = mybir.dt.float32
BF16 = mybir.dt.bfloat16
AF = mybir.ActivationFunctionType
ALU = mybir.AluOpType
BF16_NP = ml_dtypes.bfloat16

import os
M1_POOL = os.environ.get("M1_POOL", "1") == "1"

H = 128          # hidden dim
D = 128          # input dim
B = 64           # global batch
T = 1024         # sequence length
NCORES = 8
BL = B // NCORES  # per-core batch = 8
G = 4            # gates (i, f, g, o)
NCH = 2          # chains (a=fw, b=bw)
CH = 8           # window steps: CH * NCH * G * BL = 512 = one PSUM bank
NPSB = 4         # psum banks in rotation
SW = NCH * BL    # step width in h columns = 16

# deg-7 odd tanh fit on [0, BC]; the RECURRENCE c is clamped to +-BC
# off the critical path; tanh sees the unclamped c (bounded by BC*sf+1).
BC = 2.0
# coefficients (q7, q5, q3) for tanh(x) ~ x*(1 + q3 u + q5 u^2 + q7 u^3),
# constrained q1=1, fitted below in _fit_tanh7 (cached).
_TANH7 = None


def _fit_tanh_poly(Bfit, degs):
    """minimax-ish fit tanh(x) ~ x * sum_k coef_k u^k over [0,Bfit].
    degs e.g. (2,1,0) for deg-5 with free constant term."""
    x = np.linspace(1e-9, Bfit, 4001)
    t = np.tanh(x)
    u = x * x
    A = np.stack([u ** k for k in degs], axis=1)
    y = t / x
    w = np.ones_like(x)
    best = (1e9, None)
    for _ in range(120):
        coef, *_ = np.linalg.lstsq(A * (w * x)[:, None], y * w * x, rcond=None)
        err = (A @ coef - y) * x
        m = np.abs(err).max()
        if m < best[0]:
            best = (m, coef.copy())
        w = w * (0.2 + 0.8 * np.abs(err) / max(m, 1e-15))
        w /= w.max()
    return best[1], best[0]


def _fit_tanh7c(Bfit):
    """deg-7 odd, q1 constrained to 1: tanh ~ x*(1 + q3 u + q5 u^2 + q7 u^3)."""
    x = np.linspace(1e-9, Bfit, 4001)
    t = np.tanh(x)
    u = x * x
    A = np.stack([u ** 3, u ** 2, u], axis=1)
    y = t / x - 1.0
    w = np.ones_like(x)
    best = (1e9, None)
    for _ in range(120):
        coef, *_ = np.linalg.lstsq(A * (w * x)[:, None], y * w * x, rcond=None)
        err = (A @ coef - y) * x
        m = np.abs(err).max()
        if m < best[0]:
            best = (m, coef.copy())
        w = w * (0.2 + 0.8 * np.abs(err) / max(m, 1e-15))
        w /= w.max()
    return best[1], best[0]


def tanh5_consts():
    global _TANH7
    if _TANH7 is None:
        coef, err = _fit_tanh7c(BC)
        _TANH7 = (float(coef[0]), float(coef[1]), float(coef[2]))
    return _TANH7


def _rev_ap(tile_ap, col_hi_start, step_stride, nsteps, inner):
    """AP over `tile_ap`'s tensor: [[pstride,128],[-step_stride,nsteps],
    [1,inner]] starting at column col_hi_start (the FIRST emitted block)."""
    pairs = list(tile_ap.ap)
    pstride = list(pairs[0])
    return cap.AP(tensor=tile_ap.tensor,
                  offset=tile_ap.offset + col_hi_start,
                  ap=[pstride, [-step_stride, nsteps], [1, inner]])


def _fwd_ap(tile_ap, col_start, step_stride, nsteps, inner):
    pairs = list(tile_ap.ap)
    pstride = list(pairs[0])
    return cap.AP(tensor=tile_ap.tensor,
                  offset=tile_ap.offset + col_start,
                  ap=[pstride, [step_stride, nsteps], [1, inner]])


# Synchronous (in-engine-order) opcodes: their sem updates are ordered with
# the engine queue, so a later same-engine instruction is implicitly after
# them. DMA-ish opcodes complete asynchronously and must NOT count.
_SYNC_OPS = {
    "Matmult", "Ldweights", "Activation", "TensorTensor", "TensorScalarPtr",
    "TensorScalar", "ISA", "Memset", "TensorReduce", "EventSemaphore",
    "Select", "TensorCopy", "Iota", "ActTableLoad", "LoadActFuncSet",
    "ScalarTensorTensor", "TensorTensorScan", "Pool", "BnStats", "BnAggr",
}


def strip_redundant_waits(nc):
    """Remove semaphore waits that are implied by same-engine program order.

    Tile serializes dependencies touching custom-DVE (InstISA) ops with
    explicit engine-sem waits; on hardware the in-order engine pipeline
    already guarantees same-engine RAW/WAR, exactly as Tile itself assumes
    for stock-op chains. A wait (sem >= v) on engine E is redundant when
    either (a) instructions earlier in E's queue have already emitted >= v
    worth of synchronous updates to that sem, or (b) an earlier instruction
    on E already waited for >= v on that sem."""
    from collections import defaultdict
    f = nc.m.functions[0]
    nstrip = nkeep = 0
    for blk in f.blocks:
        upd = defaultdict(lambda: defaultdict(int))       # eng -> sem -> count
        waited = defaultdict(lambda: defaultdict(lambda: -1))
        insts = list(blk.instructions)
        for inst in insts:
            eng = inst.engine
            si = inst.sync_info
            if si is None:
                continue
            ow = list(si.on_wait)
            keeps = []
            for w in ow:
                strip = False
                try:
                    wm = str(w.wait_mode)
                    v = w.wait_value
                    s = w.id
                    if "ge" in wm and v is not None and eng is not None:
                        if upd[eng][s] >= v:
                            strip = True
                        else:
                            waited[eng][s] = max(waited[eng][s], v)
                except Exception:
                    pass
                if strip and inst.opcode != "EventSemaphore":
                    nstrip += 1
                else:
                    nkeep += 1
                    keeps.append(w)
            if len(keeps) != len(ow):
                si.on_wait = keeps
            for u in si.on_update:
                try:
                    um = str(u.update_mode)
                    if (("add" in um or "inc" in um)
                            and u.update_value is not None
                            and eng is not None
                            and inst.opcode in _SYNC_OPS):
                        upd[eng][u.id] += u.update_value
                except Exception:
                    pass
    return nstrip, nkeep


def build_program(t_len=T):
    nw = t_len // CH
    nc = bacc.Bacc("TRN2", target_bir_lowering=False, debug=False)

    # ---- DRAM I/O ----
    xT_d = nc.dram_tensor("xT", [D, t_len * BL], BF16, kind="ExternalInput")
    whh_d, wih_d, bias_d = {}, {}, {}
    for lay in (1, 2):
        for chn in range(NCH):
            cell = f"{'ab'[chn]}{lay}"
            whh_d[cell] = nc.dram_tensor(f"whhT_{cell}", [H, G * H], BF16,
                                         kind="ExternalInput")
            bias_d[cell] = nc.dram_tensor(f"bias_{cell}", [1, G * H], BF16,
                                          kind="ExternalInput")
            nchunk = 1 if lay == 1 else 2
            wih_d[cell] = [
                nc.dram_tensor(f"wihT_{cell}_{q}", [H, G * H], BF16,
                               kind="ExternalInput")
                for q in range(nchunk)
            ]
    o2_d = nc.dram_tensor("o2", [H, t_len * SW], BF16, kind="ExternalOutput")

    q7, q5, q3 = tanh5_consts()

    with tile.TileContext(nc) as tc:
        with tc.tile_pool(name="const", bufs=1) as const, \
             tc.tile_pool(name="ps", bufs=1, space="PSUM") as psp, \
             tc.tile_pool(name="work", bufs=4) as work:

            # ---- persistent SBUF ----
            xT = const.tile([D, t_len * BL], BF16, tag="xT")
            ndma = 8
            chunk = (t_len * BL) // ndma
            for i in range(ndma):
                nc.sync.dma_start(out=xT[:, i * chunk:(i + 1) * chunk],
                                  in_=xT_d.ap()[:, i * chunk:(i + 1) * chunk])

            whh_s, wih_s, bias_s = {}, {}, {}
            for cell in whh_d:
                whh_s[cell] = const.tile([H, G * H], BF16, name=f"whh_{cell}")
                nc.sync.dma_start(out=whh_s[cell][:, :],
                                  in_=whh_d[cell].ap()[:, :])
                bias_s[cell] = const.tile([1, G * H], BF16,
                                          name=f"bias_{cell}")
                nc.sync.dma_start(out=bias_s[cell][:, :],
                                  in_=bias_d[cell].ap()[:, :])
                wih_s[cell] = []
                for q, dd in enumerate(wih_d[cell]):
                    wt = const.tile([H, G * H], BF16, name=f"wih_{cell}_{q}")
                    nc.sync.dma_start(out=wt[:, :], in_=dd.ap()[:, :])
                    wih_s[cell].append(wt)

            ones_row = const.tile([1, CH * BL], BF16, tag="ones_row")
            nc.vector.memset(ones_row[:, :], 1.0)

            # h buffers: step-indexed, merged chains: col = k*SW + chain*BL + b
            h1 = const.tile([H, t_len * SW], BF16, tag="h1")
            h2 = const.tile([H, t_len * SW], BF16, tag="h2")

            # psum window banks
            psb = [psp.tile([H, CH * NCH * G * BL], F32, name=f"psb{j}")
                   for j in range(NPSB)]

            # step work tiles (s double-buffered to keep the sigmoid's
            # write-after-read dependency two steps back); m1/m2/cu/w come
            # from the rotating work pool per step (fresh buffers keep Tile's
            # same-engine dependency elision in play, as in chained stock ops)
            s_tt = [const.tile([H, NCH * G * BL], F32, name=f"s_t{i}")
                    for i in range(2)]
            c_t = const.tile([H, SW], F32, tag="c_t")

            def precompute_pieces(lay, j):
                """Return emit-thunks for window j's input projection (both
                chains), as a list of small matmuls."""
                ps = psb[j % NPSB]
                pieces = []
                first = [True]
                base_k = j * CH

                def out_ap(chn, g):
                    # columns slot*64 + g*16 + chn*8 + b over slots 0..CH-1
                    return _fwd_ap(ps[:, :], g * (NCH * BL) + chn * BL,
                                   NCH * G * BL, CH, BL)

                for chn in range(NCH):
                    cell = f"{'ab'[chn]}{lay}"
                    if lay == 1:
                        if chn == 0:
                            rhs = [_fwd_ap(xT[:, :], base_k * BL, BL, CH, BL)]
                        else:
                            rhs = [_rev_ap(xT[:, :], (t_len - 1 - base_k) * BL,
                                           BL, CH, BL)]
                    else:
                        if chn == 0:
                            rhs = [
                                _fwd_ap(h1[:, :], base_k * SW, SW, CH, BL),
                                _rev_ap(h1[:, :],
                                        (t_len - 1 - base_k) * SW + BL,
                                        SW, CH, BL),
                            ]
                        else:
                            rhs = [
                                _rev_ap(h1[:, :], (t_len - 1 - base_k) * SW,
                                        SW, CH, BL),
                                _fwd_ap(h1[:, :], base_k * SW + BL,
                                        SW, CH, BL),
                            ]
                    for g in range(G):
                        for q, r in enumerate(rhs):
                            def mk(cell=cell, g=g, q=q, r=r, chn=chn):
                                st = first[0]
                                first[0] = False
                                nc.tensor.matmul(
                                    out_ap(chn, g),
                                    wih_s[cell][q][:, g * H:(g + 1) * H],
                                    r, start=st, stop=False)
                            pieces.append(mk)
                    # bias via K=1 rank-1 matmul
                    for g in range(G):
                        last = (chn == NCH - 1 and g == G - 1)
                        def mkb(cell=cell, g=g, chn=chn, last=last):
                            nc.tensor.matmul(
                                out_ap(chn, g),
                                bias_s[cell][:, g * H:(g + 1) * H],
                                ones_row[:, :], start=False, stop=last)
                        pieces.append(mkb)
                return pieces

            def s_slice(s_t, g):
                # gate-major: s_t columns g*16 + chain*8 + b -> [128, 16]
                return s_t[:, g * SW:(g + 1) * SW]

            def v16(tl):
                return tl[:, :]

            for lay in (1, 2):
                hout = h1 if lay == 1 else h2
                hin = h1  # layer2 recurrent input is its own hout; hin unused
                nc.vector.memset(c_t[:, :], 0.0)

                # prime first NPRE windows
                NPRE = 2
                pending = []
                for j in range(min(NPRE, nw)):
                    with tc.high_priority(offset=-1_000_000):
                        for p in precompute_pieces(lay, j):
                            p()

                for k in range(t_len):
                    j, slot = k // CH, k % CH
                    ps = psb[j % NPSB]
                    # recurrent matmuls (skip at k=0: h_prev = 0)
                    if k > 0:
                        hprev = hout[:, (k - 1) * SW:k * SW]
                        for chn in range(NCH):
                            cell = f"{'ab'[chn]}{lay}"
                            for g in range(G):
                                base = (slot * (NCH * G * BL)
                                        + g * (NCH * BL) + chn * BL)
                                nc.tensor.matmul(
                                    ps[:, base:base + BL],
                                    whh_s[cell][:, g * H:(g + 1) * H],
                                    hprev[:, chn * BL:(chn + 1) * BL],
                                    start=False, stop=False,
                                    skip_group_check=True)

                    # sigmoid over the whole slot (both chains, 4 gates)
                    s_t = s_tt[k % 2]
                    nc.scalar.activation(
                        s_t[:, :],
                        ps[:, slot * (NCH * G * BL):
                           (slot + 1) * (NCH * G * BL)],
                        AF.Sigmoid)

                    # all-stock chain: m1, m2, cu, w back-to-back on DVE
                    m1_t = work.tile([H, SW], F32, name="m1")
                    m2_t = work.tile([H, SW], F32, name="m2")
                    cu_t = work.tile([H, SW], F32, name="cu")
                    w_t = work.tile([H, SW], F32, name="w")
                    # m1 = sigma_f * c_prev (clipped c)
                    nc.vector.tensor_tensor(v16(m1_t), s_slice(s_t, 1),
                                            v16(c_t), ALU.mult)
                    # m2' = (sigma(2 zg) - 0.5) * sigma_i   [= tanh(zg)*si/2]
                    nc.vector.scalar_tensor_tensor(
                        v16(m2_t), s_slice(s_t, 2), 0.5, s_slice(s_t, 0),
                        ALU.subtract, ALU.mult)
                    # cu = 2*m2' + m1 (unclipped; bounded by BC*sf+1 < 3)
                    nc.vector.scalar_tensor_tensor(
                        v16(cu_t), v16(m2_t), 2.0, v16(m1_t),
                        ALU.mult, ALU.add)
                    # w = cu * sigma_o
                    nc.vector.tensor_tensor(v16(w_t), v16(cu_t),
                                            s_slice(s_t, 3), ALU.mult)
                    # c = clip(cu) for the recurrence (off critical path)
                    nc.vector.tensor_scalar(v16(c_t), v16(cu_t), BC, -BC,
                                            ALU.min, ALU.max)
                    # h = tanh_deg7(cu) * sigma_o   [single custom hop]
                    nc.vector._custom_dve(
                        TANHH, out=v16(hout[:, k * SW:(k + 1) * SW]),
                        in0=v16(cu_t), in1=v16(w_t),
                        s0=q7, s1=q5, imm2=q3)

                    # spread next-window precompute across this window's steps
                    if slot == 0 and j + NPRE < nw:
                        pending = precompute_pieces(lay, j + NPRE)
                    if pending:
                        npiece = (len(pending) + CH - 1 - slot) // (CH - slot) \
                            if slot < CH else len(pending)
                        with tc.high_priority(offset=-1_000_000):
                            for _ in range(npiece):
                                if pending:
                                    pending.pop(0)()

                    if lay == 2:
                        ock = t_len // 8
                        if (k + 1) % ock == 0:
                            ci = k // ock
                            nc.sync.dma_start(
                                out=o2_d.ap()[:, ci * ock * SW:
                                              (ci + 1) * ock * SW],
                                in_=h2[:, ci * ock * SW:(ci + 1) * ock * SW])

    nc.compile()
    return nc


# ---------------------------------------------------------------------------
# Host-side packing
# ---------------------------------------------------------------------------

def _prep_weights(Wih, Whh, bih, bhh):
    """Transpose, g-gate x2 scaling, bf16."""
    gscale = np.array([1.0, 1.0, 2.0, 1.0], np.float32)
    wihT = np.ascontiguousarray(Wih.T).astype(np.float32)      # [in, 4H]
    whhT = np.ascontiguousarray(Whh.T).astype(np.float32)      # [H, 4H]
    bias = (bih + bhh).astype(np.float32)                      # [4H]
    for g in range(G):
        sl = slice(g * H, (g + 1) * H)
        wihT[:, sl] *= gscale[g]
        whhT[:, sl] *= gscale[g]
        bias[sl] *= gscale[g]
    nq = wihT.shape[0] // H
    wih_chunks = [np.ascontiguousarray(wihT[q * H:(q + 1) * H]).astype(BF16_NP)
                  for q in range(nq)]
    bias_row = bias.reshape(1, G * H).astype(BF16_NP)
    return wih_chunks, whhT.astype(BF16_NP), bias_row


def core_xT(xs, t_len):
    """Per-core input: xs [BL, T, D] -> xT [D, T*BL] (time-major, fw order)."""
    return np.ascontiguousarray(
        xs.transpose(2, 1, 0).reshape(D, t_len * BL)).astype(BF16_NP)


def core_gather(res_c, t_len):
    """Per-core output: o2 [H, T*SW] step-indexed -> [BL, T, 2H] f32."""
    o = np.asarray(res_c["o2"]).astype(np.float32)
    o = o.reshape(H, t_len, NCH, BL)
    out = np.empty((BL, t_len, 2 * H), np.float32)
    # chain a: step k == time tau
    out[:, :, :H] = o[:, :, 0, :].transpose(2, 1, 0)
    # chain b: step k == time T-1-k -> reverse time axis
    out[:, :, H:] = o[:, ::-1, 1, :].transpose(2, 1, 0)
    return out


_PROG_CACHE = {}


def prepare_in_maps(x, kw):
    x = np.asarray(x, np.float32)
    t_len = x.shape[1]
    cells = {"a1": (kw["Wih_fw1"], kw["Whh_fw1"], kw["bih_fw1"], kw["bhh_fw1"]),
             "b1": (kw["Wih_bw1"], kw["Whh_bw1"], kw["bih_bw1"], kw["bhh_bw1"]),
             "a2": (kw["Wih_fw2"], kw["Whh_fw2"], kw["bih_fw2"], kw["bhh_fw2"]),
             "b2": (kw["Wih_bw2"], kw["Whh_bw2"], kw["bih_bw2"], kw["bhh_bw2"])}
    wmaps = {}
    for cell, (Wih, Whh, bih, bhh) in cells.items():
        wih_chunks, whhT, bias_row = _prep_weights(
            np.asarray(Wih, np.float32), np.asarray(Whh, np.float32),
            np.asarray(bih, np.float32), np.asarray(bhh, np.float32))
        wmaps[f"whhT_{cell}"] = whhT
        wmaps[f"bias_{cell}"] = bias_row
        for q, wc in enumerate(wih_chunks):
            wmaps[f"wihT_{cell}_{q}"] = wc

    core_ids = list(range(NCORES))
    in_maps = []
    for c in core_ids:
        m = {"xT": core_xT(x[c * BL:(c + 1) * BL], t_len)}
        m.update(wmaps)
        in_maps.append(m)
    return in_maps, core_ids


def kernel(x, lengths, **kw):
    x = np.asarray(x, np.float32)
    t_len = x.shape[1]
    in_maps, core_ids = prepare_in_maps(x, kw)
    if t_len not in _PROG_CACHE:
        _PROG_CACHE[t_len] = build_program(t_len)
    nc = _PROG_CACHE[t_len]
    return _execute(nc, in_maps, core_ids, t_len)[0]


def _execute(nc, in_maps, core_ids, t_len, **run_kwargs):
    r = run_bass_kernel_spmd(nc, in_maps, core_ids, **run_kwargs)
    out = np.empty((B, t_len, 2 * H), np.float32)
    for c in core_ids:
        out[c * BL:(c + 1) * BL] = core_gather(r.results[c], t_len)
    return out, r


# revision 16
# speedup vs baseline: 1.2735x; 1.1231x over previous
"""Bass/Trainium2 kernel for a 2-layer bidirectional LSTM (CustomBiLSTM).

Strategy: data-parallel over batch across 8 NeuronCores (B=64 -> 8 per core).
Per core, each layer runs its forward and backward chains MERGED: both
chains are step-indexed (chain b's step k processes original time T-1-k, with
all time reversal handled by negative-stride access patterns), so every
per-step instruction covers both chains at once:

  - 8 recurrent matmuls (2 chains x 4 gates) accumulate into one PSUM bank
    laid out [slot(8), chain(2), gate(4), batch(8)] = 512 f32 columns.
  - ONE Activation instruction applies exact sigmoid to the whole 64-column
    slot (g-gate weights pre-scaled by 2: sigma(2z) = (tanh(z)+1)/2).
  - A short chain of Vector-engine ops (same engine -> no semaphore hops)
    computes the cell update, including two custom DVE ops:
      CFMA:    c = clip(2*m2 + m1, -Bc, Bc)
      TANH_P7: tanh(c) as a degree-7 odd Horner polynomial (c pre-clamped)
    and writes h (bf16) into a step-indexed h buffer.

The critical cycle per step is matmul -> sigmoid -> 5 DVE ops -> matmul.
Input projections (Wih @ x + bias) are precomputed into PSUM windows of 8
steps, emitted in small pieces spread across the previous window's steps so
the in-order PE queue never stalls a recurrent matmul.
"""

import numpy as np
import ml_dtypes

try:
    import concourse.bass as bass
except ImportError:
    import sys
    sys.path.insert(0, "/opt/trn_rl_repo")
    import concourse.bass as bass

import concourse.bacc as bacc
import concourse.tile as tile
import concourse.ap as cap
from concourse import mybir
from concourse.bass_utils import run_bass_kernel_spmd

# ---------------------------------------------------------------------------
# Custom DVE op registration
# ---------------------------------------------------------------------------
from concourse.dve_ops import DveOp, OPS, CUSTOM_DVE_SPECS, _SUB_OPCODE_FOR_NAME
from concourse.dve_spec import (
    Spec, Src0, Src1, C0, C1, C2, Zero, One, maxx, minn, lower, _has_src1,
)
from concourse.dve_uop import DveOpSpec


def _register(name, spec):
    if name in _SUB_OPCODE_FOR_NAME:
        return next(o for o in OPS if o.name == name)
    row = max(_SUB_OPCODE_FOR_NAME.values()) + 1
    assert row < 0x20, "custom DVE opcode rows exhausted"
    _SUB_OPCODE_FOR_NAME[name] = row
    shas = {}
    for ver in ("v3", "v4"):
        try:
            tmp = DveOpSpec(name=name, opcode=row, uops=lower(spec, ver=ver),
                            rd1_en=_has_src1(spec))
            shas[ver] = tmp.sha(ver)
        except Exception:
            pass
    op = DveOp(name, spec, subdim=False, uops_sha=shas)
    OPS.append(op)
    CUSTOM_DVE_SPECS[name] = spec
    return op


# c = clip(in0*s0 + in1, -s1, s1)
CFMA = _register(
    "ANT_CFMA",
    Spec(body=maxx(minn(Src0 * C0 + Src1, C1), Zero - C1),
         reference=lambda in0, in1, s0, s1, imm2:
         np.clip(in0 * s0 + in1, -s1, s1)),
)

# tanh(x) ~= x * (1 + s1... deg-7 odd Horner: ((s0*u + s1)*u + imm2)*u + 1
_u = Src0 * Src0
TANH_P7 = _register(
    "ANT_TANH_P7",
    Spec(body=((((C0 * _u + C1) * _u + C2) * _u) + One) * Src0,
         reference=lambda in0, s0, s1, imm2:
         ((((s0 * (in0 * in0) + s1) * (in0 * in0) + imm2) * (in0 * in0))
          + 1.0) * in0),
)

# h = tanh_deg5(in0) * in1: out = (((s0*u + s1)*u + imm2) * in0) * in1
D1H = _register(
    "ANT_D1H",
    Spec(body=(((C0 * _u + C1) * _u + C2) * Src0) * Src1,
         reference=lambda in0, in1, s0, s1, imm2:
         (((s0 * (in0 * in0) + s1) * (in0 * in0) + imm2) * in0) * in1),
)

# h = tanh_deg7(c) * sigma_o with w := c*sigma_o passed as in1:
#   out = ((((s0*u + s1)*u + imm2)*u) + 1) * in1,  u = in0^2
# (equals x*(1 + q3 u + q5 u^2 + q7 u^3) * sigma_o since in1 = x*sigma_o)
TANHH = _register(
    "ANT_TANHH",
    Spec(body=((((C0 * _u + C1) * _u + C2) * _u) + One) * Src1,
         reference=lambda in0, in1, s0, s1, imm2:
         ((((s0 * (in0 * in0) + s1) * (in0 * in0) + imm2) * (in0 * in0))
          + 1.0) * in1),
)

# h = tanh(c)*sigma_o via monic-factored deg-7 odd poly evaluated on the
# beta-pre-scaled cell state x' = beta*c (beta^7 = q7 < 0):
#   out = (x'^2 + s0) * ((x'^2 + s1) * x'^2 + imm2) * x' * in1
TANHH_M = _register(
    "ANT_TANHH_M",
    Spec(body=(((_u + C0) * ((_u + C1) * _u + C2)) * Src0) * Src1,
         reference=lambda in0, in1, s0, s1, imm2:
         ((((in0 * in0) + s0) * (((in0 * in0) + s1) * (in0 * in0) + imm2))
          * in0) * in1),
)

F32 = mybir.dt.float32
BF16 = mybir.dt.bfloat16
AF = mybir.ActivationFunctionType
ALU = mybir.AluOpType
BF16_NP = ml_dtypes.bfloat16

import os
M1_POOL = os.environ.get("M1_POOL", "1") == "1"

H = 128          # hidden dim
D = 128          # input dim
B = 64           # global batch
T = 1024         # sequence length
NCORES = 8
BL = B // NCORES  # per-core batch = 8
G = 4            # gates (i, f, g, o)
NCH = 2          # chains (a=fw, b=bw)
CH = 8           # window steps: CH * NCH * G * BL = 512 = one PSUM bank
NPSB = 4         # psum banks in rotation
SW = NCH * BL    # step width in h columns = 16

# deg-7 odd tanh fit on [0, BC]; the RECURRENCE c is clamped to +-BC
# off the critical path; tanh sees the unclamped c (bounded by BC*sf+1).
BC = 2.0
# coefficients (q7, q5, q3) for tanh(x) ~ x*(1 + q3 u + q5 u^2 + q7 u^3),
# constrained q1=1, fitted below in _fit_tanh7 (cached).
_TANH7 = None


def _fit_tanh_poly(Bfit, degs):
    """minimax-ish fit tanh(x) ~ x * sum_k coef_k u^k over [0,Bfit].
    degs e.g. (2,1,0) for deg-5 with free constant term."""
    x = np.linspace(1e-9, Bfit, 4001)
    t = np.tanh(x)
    u = x * x
    A = np.stack([u ** k for k in degs], axis=1)
    y = t / x
    w = np.ones_like(x)
    best = (1e9, None)
    for _ in range(120):
        coef, *_ = np.linalg.lstsq(A * (w * x)[:, None], y * w * x, rcond=None)
        err = (A @ coef - y) * x
        m = np.abs(err).max()
        if m < best[0]:
            best = (m, coef.copy())
        w = w * (0.2 + 0.8 * np.abs(err) / max(m, 1e-15))
        w /= w.max()
    return best[1], best[0]


def _fit_tanh7c(Bfit):
    """deg-7 odd, q1 constrained to 1: tanh ~ x*(1 + q3 u + q5 u^2 + q7 u^3)."""
    x = np.linspace(1e-9, Bfit, 4001)
    t = np.tanh(x)
    u = x * x
    A = np.stack([u ** 3, u ** 2, u], axis=1)
    y = t / x - 1.0
    w = np.ones_like(x)
    best = (1e9, None)
    for _ in range(120):
        coef, *_ = np.linalg.lstsq(A * (w * x)[:, None], y * w * x, rcond=None)
        err = (A @ coef - y) * x
        m = np.abs(err).max()
        if m < best[0]:
            best = (m, coef.copy())
        w = w * (0.2 + 0.8 * np.abs(err) / max(m, 1e-15))
        w /= w.max()
    return best[1], best[0]


def tanh5_consts():
    global _TANH7
    if _TANH7 is None:
        coef, err = _fit_tanh7c(BC)
        _TANH7 = (float(coef[0]), float(coef[1]), float(coef[2]))
    return _TANH7


_TANHM = None


def tanhm_consts():
    """(beta, C0, C1, C2) for the monic-factored full-dof deg-7 fit."""
    global _TANHM
    if _TANHM is None:
        coef, err = _fit_tanh_poly(BC, (3, 2, 1, 0))
        q7, q5, q3, q1 = [float(v) for v in coef]
        assert q7 < 0
        beta = -((-q7) ** (1.0 / 7.0))
        A = q5 / beta ** 5
        Bc_ = q3 / beta ** 3
        Cc = q1 / beta
        roots = np.roots([1.0, A, Bc_, Cc])
        real = sorted(r.real for r in roots if abs(r.imag) < 1e-9)
        v0 = real[0]
        _TANHM = (beta, float(-v0), float(A + v0), float(-Cc / v0))
    return _TANHM


def _rev_ap(tile_ap, col_hi_start, step_stride, nsteps, inner):
    """AP over `tile_ap`'s tensor: [[pstride,128],[-step_stride,nsteps],
    [1,inner]] starting at column col_hi_start (the FIRST emitted block)."""
    pairs = list(tile_ap.ap)
    pstride = list(pairs[0])
    return cap.AP(tensor=tile_ap.tensor,
                  offset=tile_ap.offset + col_hi_start,
                  ap=[pstride, [-step_stride, nsteps], [1, inner]])


def _fwd_ap(tile_ap, col_start, step_stride, nsteps, inner):
    pairs = list(tile_ap.ap)
    pstride = list(pairs[0])
    return cap.AP(tensor=tile_ap.tensor,
                  offset=tile_ap.offset + col_start,
                  ap=[pstride, [step_stride, nsteps], [1, inner]])


# Synchronous (in-engine-order) opcodes: their sem updates are ordered with
# the engine queue, so a later same-engine instruction is implicitly after
# them. DMA-ish opcodes complete asynchronously and must NOT count.
_SYNC_OPS = {
    "Matmult", "Ldweights", "Activation", "TensorTensor", "TensorScalarPtr",
    "TensorScalar", "ISA", "Memset", "TensorReduce", "EventSemaphore",
    "Select", "TensorCopy", "Iota", "ActTableLoad", "LoadActFuncSet",
    "ScalarTensorTensor", "TensorTensorScan", "Pool", "BnStats", "BnAggr",
}


def strip_redundant_waits(nc):
    """Remove semaphore waits that are implied by same-engine program order.

    Tile serializes dependencies touching custom-DVE (InstISA) ops with
    explicit engine-sem waits; on hardware the in-order engine pipeline
    already guarantees same-engine RAW/WAR, exactly as Tile itself assumes
    for stock-op chains. A wait (sem >= v) on engine E is redundant when
    either (a) instructions earlier in E's queue have already emitted >= v
    worth of synchronous updates to that sem, or (b) an earlier instruction
    on E already waited for >= v on that sem."""
    from collections import defaultdict
    f = nc.m.functions[0]
    nstrip = nkeep = 0
    for blk in f.blocks:
        upd = defaultdict(lambda: defaultdict(int))       # eng -> sem -> count
        waited = defaultdict(lambda: defaultdict(lambda: -1))
        insts = list(blk.instructions)
        for inst in insts:
            eng = inst.engine
            si = inst.sync_info
            if si is None:
                continue
            ow = list(si.on_wait)
            keeps = []
            for w in ow:
                strip = False
                try:
                    wm = str(w.wait_mode)
                    v = w.wait_value
                    s = w.id
                    if "ge" in wm and v is not None and eng is not None:
                        if upd[eng][s] >= v:
                            strip = True
                        else:
                            waited[eng][s] = max(waited[eng][s], v)
                except Exception:
                    pass
                if strip and inst.opcode != "EventSemaphore":
                    nstrip += 1
                else:
                    nkeep += 1
                    keeps.append(w)
            if len(keeps) != len(ow):
                si.on_wait = keeps
            for u in si.on_update:
                try:
                    um = str(u.update_mode)
                    if (("add" in um or "inc" in um)
                            and u.update_value is not None
                            and eng is not None
                            and inst.opcode in _SYNC_OPS):
                        upd[eng][u.id] += u.update_value
                except Exception:
                    pass
    return nstrip, nkeep


def build_program(t_len=T):
    nw = t_len // CH
    nc = bacc.Bacc("TRN2", target_bir_lowering=False, debug=False)

    # ---- DRAM I/O ----
    xT_d = nc.dram_tensor("xT", [D, t_len * BL], BF16, kind="ExternalInput")
    whh_d, wih_d, bias_d = {}, {}, {}
    for lay in (1, 2):
        for chn in range(NCH):
            cell = f"{'ab'[chn]}{lay}"
            whh_d[cell] = nc.dram_tensor(f"whhT_{cell}", [H, G * H], BF16,
                                         kind="ExternalInput")
            bias_d[cell] = nc.dram_tensor(f"bias_{cell}", [1, G * H], BF16,
                                          kind="ExternalInput")
            nchunk = 1 if lay == 1 else 2
            wih_d[cell] = [
                nc.dram_tensor(f"wihT_{cell}_{q}", [H, G * H], BF16,
                               kind="ExternalInput")
                for q in range(nchunk)
            ]
    o2_d = nc.dram_tensor("o2", [H, t_len * SW], BF16, kind="ExternalOutput")

    beta, mc0, mc1, mc2 = tanhm_consts()

    with tile.TileContext(nc) as tc:
        with tc.tile_pool(name="const", bufs=1) as const, \
             tc.tile_pool(name="ps", bufs=1, space="PSUM") as psp, \
             tc.tile_pool(name="work", bufs=4) as work:

            # ---- persistent SBUF ----
            xT = const.tile([D, t_len * BL], BF16, tag="xT")
            ndma = 8
            chunk = (t_len * BL) // ndma
            for i in range(ndma):
                nc.sync.dma_start(out=xT[:, i * chunk:(i + 1) * chunk],
                                  in_=xT_d.ap()[:, i * chunk:(i + 1) * chunk])

            whh_s, wih_s, bias_s = {}, {}, {}
            for cell in whh_d:
                whh_s[cell] = const.tile([H, G * H], BF16, name=f"whh_{cell}")
                nc.sync.dma_start(out=whh_s[cell][:, :],
                                  in_=whh_d[cell].ap()[:, :])
                bias_s[cell] = const.tile([1, G * H], BF16,
                                          name=f"bias_{cell}")
                nc.sync.dma_start(out=bias_s[cell][:, :],
                                  in_=bias_d[cell].ap()[:, :])
                wih_s[cell] = []
                for q, dd in enumerate(wih_d[cell]):
                    wt = const.tile([H, G * H], BF16, name=f"wih_{cell}_{q}")
                    nc.sync.dma_start(out=wt[:, :], in_=dd.ap()[:, :])
                    wih_s[cell].append(wt)

            ones_row = const.tile([1, CH * BL], BF16, tag="ones_row")
            nc.vector.memset(ones_row[:, :], 1.0)

            # h buffers: step-indexed, merged chains: col = k*SW + chain*BL + b
            h1 = const.tile([H, t_len * SW], BF16, tag="h1")
            h2 = const.tile([H, t_len * SW], BF16, tag="h2")

            # psum window banks
            psb = [psp.tile([H, CH * NCH * G * BL], F32, name=f"psb{j}")
                   for j in range(NPSB)]

            # step work tiles (s double-buffered to keep the sigmoid's
            # write-after-read dependency two steps back); m1/m2/cu/w come
            # from the rotating work pool per step (fresh buffers keep Tile's
            # same-engine dependency elision in play, as in chained stock ops)
            s_tt = [const.tile([H, NCH * G * BL], F32, name=f"s_t{i}")
                    for i in range(2)]
            c_t = const.tile([H, SW], F32, tag="c_t")

            def precompute_pieces(lay, j):
                """Return emit-thunks for window j's input projection (both
                chains), as a list of small matmuls."""
                ps = psb[j % NPSB]
                pieces = []
                first = [True]
                base_k = j * CH

                def out_ap(chn, g):
                    # columns slot*64 + g*16 + chn*8 + b over slots 0..CH-1
                    return _fwd_ap(ps[:, :], g * (NCH * BL) + chn * BL,
                                   NCH * G * BL, CH, BL)

                for chn in range(NCH):
                    cell = f"{'ab'[chn]}{lay}"
                    if lay == 1:
                        if chn == 0:
                            rhs = [_fwd_ap(xT[:, :], base_k * BL, BL, CH, BL)]
                        else:
                            rhs = [_rev_ap(xT[:, :], (t_len - 1 - base_k) * BL,
                                           BL, CH, BL)]
                    else:
                        if chn == 0:
                            rhs = [
                                _fwd_ap(h1[:, :], base_k * SW, SW, CH, BL),
                                _rev_ap(h1[:, :],
                                        (t_len - 1 - base_k) * SW + BL,
                                        SW, CH, BL),
                            ]
                        else:
                            rhs = [
                                _rev_ap(h1[:, :], (t_len - 1 - base_k) * SW,
                                        SW, CH, BL),
                                _fwd_ap(h1[:, :], base_k * SW + BL,
                                        SW, CH, BL),
                            ]
                    for g in range(G):
                        for q, r in enumerate(rhs):
                            def mk(cell=cell, g=g, q=q, r=r, chn=chn):
                                st = first[0]
                                first[0] = False
                                nc.tensor.matmul(
                                    out_ap(chn, g),
                                    wih_s[cell][q][:, g * H:(g + 1) * H],
                                    r, start=st, stop=False)
                            pieces.append(mk)
                    # bias via K=1 rank-1 matmul
                    for g in range(G):
                        last = (chn == NCH - 1 and g == G - 1)
                        def mkb(cell=cell, g=g, chn=chn, last=last):
                            nc.tensor.matmul(
                                out_ap(chn, g),
                                bias_s[cell][:, g * H:(g + 1) * H],
                                ones_row[:, :], start=False, stop=last)
                        pieces.append(mkb)
                return pieces

            def s_slice(s_t, g):
                # gate-major: s_t columns g*16 + chain*8 + b -> [128, 16]
                return s_t[:, g * SW:(g + 1) * SW]

            def v16(tl):
                return tl[:, :]

            for lay in (1, 2):
                hout = h1 if lay == 1 else h2
                hin = h1  # layer2 recurrent input is its own hout; hin unused
                nc.vector.memset(c_t[:, :], 0.0)

                # prime first NPRE windows
                NPRE = 2
                pending = []
                for j in range(min(NPRE, nw)):
                    with tc.high_priority(offset=-1_000_000):
                        for p in precompute_pieces(lay, j):
                            p()

                for k in range(t_len):
                    j, slot = k // CH, k % CH
                    ps = psb[j % NPSB]
                    # recurrent matmuls (skip at k=0: h_prev = 0)
                    if k > 0:
                        hprev = hout[:, (k - 1) * SW:k * SW]
                        for chn in range(NCH):
                            cell = f"{'ab'[chn]}{lay}"
                            for g in range(G):
                                base = (slot * (NCH * G * BL)
                                        + g * (NCH * BL) + chn * BL)
                                nc.tensor.matmul(
                                    ps[:, base:base + BL],
                                    whh_s[cell][:, g * H:(g + 1) * H],
                                    hprev[:, chn * BL:(chn + 1) * BL],
                                    start=False, stop=False,
                                    skip_group_check=True)

                    # sigmoid over the whole slot (both chains, 4 gates)
                    s_t = s_tt[k % 2]
                    nc.scalar.activation(
                        s_t[:, :],
                        ps[:, slot * (NCH * G * BL):
                           (slot + 1) * (NCH * G * BL)],
                        AF.Sigmoid)

                    # all-stock chain, beta-scaled cell state c' = beta*c
                    m1_t = work.tile([H, SW], F32, name="m1")
                    m2_t = work.tile([H, SW], F32, name="m2")
                    cu_t = work.tile([H, SW], F32, name="cu")
                    # m1 = sigma_f * c'_prev (clipped)
                    nc.vector.tensor_tensor(v16(m1_t), s_slice(s_t, 1),
                                            v16(c_t), ALU.mult)
                    # m2' = (sigma(2 zg) - 0.5) * sigma_i   [= tanh(zg)*si/2]
                    nc.vector.scalar_tensor_tensor(
                        v16(m2_t), s_slice(s_t, 2), 0.5, s_slice(s_t, 0),
                        ALU.subtract, ALU.mult)
                    # cu' = 2*beta*m2' + m1 (unclipped, beta-units)
                    nc.vector.scalar_tensor_tensor(
                        v16(cu_t), v16(m2_t), 2.0 * beta, v16(m1_t),
                        ALU.mult, ALU.add)
                    # h = tanh(c)*sigma_o via monic deg-7 [single custom hop]
                    nc.vector._custom_dve(
                        TANHH_M, out=v16(hout[:, k * SW:(k + 1) * SW]),
                        in0=v16(cu_t), in1=s_slice(s_t, 3),
                        s0=mc0, s1=mc1, imm2=mc2)
                    # c' = clip(cu') for the recurrence (off critical path)
                    nc.vector.tensor_scalar(v16(c_t), v16(cu_t),
                                            abs(beta) * BC, -abs(beta) * BC,
                                            ALU.min, ALU.max)

                    # spread next-window precompute across this window's steps
                    if slot == 0 and j + NPRE < nw:
                        pending = precompute_pieces(lay, j + NPRE)
                    if pending:
                        npiece = (len(pending) + CH - 1 - slot) // (CH - slot) \
                            if slot < CH else len(pending)
                        with tc.high_priority(offset=-1_000_000):
                            for _ in range(npiece):
                                if pending:
                                    pending.pop(0)()

                    if lay == 2:
                        ock = t_len // 8
                        if (k + 1) % ock == 0:
                            ci = k // ock
                            nc.sync.dma_start(
                                out=o2_d.ap()[:, ci * ock * SW:
                                              (ci + 1) * ock * SW],
                                in_=h2[:, ci * ock * SW:(ci + 1) * ock * SW])

    nc.compile()
    return nc


# ---------------------------------------------------------------------------
# Host-side packing
# ---------------------------------------------------------------------------

def _prep_weights(Wih, Whh, bih, bhh):
    """Transpose, g-gate x2 scaling, bf16."""
    gscale = np.array([1.0, 1.0, 2.0, 1.0], np.float32)
    wihT = np.ascontiguousarray(Wih.T).astype(np.float32)      # [in, 4H]
    whhT = np.ascontiguousarray(Whh.T).astype(np.float32)      # [H, 4H]
    bias = (bih + bhh).astype(np.float32)                      # [4H]
    for g in range(G):
        sl = slice(g * H, (g + 1) * H)
        wihT[:, sl] *= gscale[g]
        whhT[:, sl] *= gscale[g]
        bias[sl] *= gscale[g]
    nq = wihT.shape[0] // H
    wih_chunks = [np.ascontiguousarray(wihT[q * H:(q + 1) * H]).astype(BF16_NP)
                  for q in range(nq)]
    bias_row = bias.reshape(1, G * H).astype(BF16_NP)
    return wih_chunks, whhT.astype(BF16_NP), bias_row


def core_xT(xs, t_len):
    """Per-core input: xs [BL, T, D] -> xT [D, T*BL] (time-major, fw order)."""
    return np.ascontiguousarray(
        xs.transpose(2, 1, 0).reshape(D, t_len * BL)).astype(BF16_NP)


def core_gather(res_c, t_len):
    """Per-core output: o2 [H, T*SW] step-indexed -> [BL, T, 2H] f32."""
    o = np.asarray(res_c["o2"]).astype(np.float32)
    o = o.reshape(H, t_len, NCH, BL)
    out = np.empty((BL, t_len, 2 * H), np.float32)
    # chain a: step k == time tau
    out[:, :, :H] = o[:, :, 0, :].transpose(2, 1, 0)
    # chain b: step k == time T-1-k -> reverse time axis
    out[:, :, H:] = o[:, ::-1, 1, :].transpose(2, 1, 0)
    return out


_PROG_CACHE = {}


def prepare_in_maps(x, kw):
    x = np.asarray(x, np.float32)
    t_len = x.shape[1]
    cells = {"a1": (kw["Wih_fw1"], kw["Whh_fw1"], kw["bih_fw1"], kw["bhh_fw1"]),
             "b1": (kw["Wih_bw1"], kw["Whh_bw1"], kw["bih_bw1"], kw["bhh_bw1"]),
             "a2": (kw["Wih_fw2"], kw["Whh_fw2"], kw["bih_fw2"], kw["bhh_fw2"]),
             "b2": (kw["Wih_bw2"], kw["Whh_bw2"], kw["bih_bw2"], kw["bhh_bw2"])}
    wmaps = {}
    for cell, (Wih, Whh, bih, bhh) in cells.items():
        wih_chunks, whhT, bias_row = _prep_weights(
            np.asarray(Wih, np.float32), np.asarray(Whh, np.float32),
            np.asarray(bih, np.float32), np.asarray(bhh, np.float32))
        wmaps[f"whhT_{cell}"] = whhT
        wmaps[f"bias_{cell}"] = bias_row
        for q, wc in enumerate(wih_chunks):
            wmaps[f"wihT_{cell}_{q}"] = wc

    core_ids = list(range(NCORES))
    in_maps = []
    for c in core_ids:
        m = {"xT": core_xT(x[c * BL:(c + 1) * BL], t_len)}
        m.update(wmaps)
        in_maps.append(m)
    return in_maps, core_ids


def kernel(x, lengths, **kw):
    x = np.asarray(x, np.float32)
    t_len = x.shape[1]
    in_maps, core_ids = prepare_in_maps(x, kw)
    if t_len not in _PROG_CACHE:
        _PROG_CACHE[t_len] = build_program(t_len)
    nc = _PROG_CACHE[t_len]
    return _execute(nc, in_maps, core_ids, t_len)[0]


def _execute(nc, in_maps, core_ids, t_len, **run_kwargs):
    r = run_bass_kernel_spmd(nc, in_maps, core_ids, **run_kwargs)
    out = np.empty((B, t_len, 2 * H), np.float32)
    for c in core_ids:
        out[c * BL:(c + 1) * BL] = core_gather(r.results[c], t_len)
    return out, r
